# revision 1
# baseline (speedup 1.0000x reference)
"""3-layer custom GRU (original-paper variant, reset applied before the
hidden matmul) on 8 trn2 NeuronCores.

Strategy: data-parallel over batch (16 rows/core), zero collectives (the
measured per-collective cost on this stack is ~340us, which rules out any
model-parallel scheme needing per-timestep gathers). Each core runs the
full 3-layer stack on its batch shard, layer-sequentially: first a bulk
matmul computes gi_l = X_l @ Wih_l^T for all timesteps at once (X_l is x
for layer 0, else the previous layer's hidden series, kept in SBUF), then
the sequential recurrence runs over t with Whh_l^T resident in SBUF as
bf16 (fp32 psum accumulation, fp32 hidden state).

The recurrence is weight-load bound on the PE (a 128x128 bf16 stationary
block load ~53ns dominates the 16-column moving stream), so wall time is
~(Whh elements)/(128 lanes * 2.4GHz) per step regardless of batch width —
which is why replicating weights and splitting batch 8 ways is the right
trade on this machine. Layer 2's Whh^T (25.2MB bf16) exceeds the 24MB
SBUF, so its leading columns stay resident and the tail streams from HBM
every step through a small rotating buffer.

gi is staged in DRAM as [T, 16, 3H] bf16 — written contiguously by a
[(t,b), gate]-layout bulk matmul, read contiguously per step, and
transposed into [gate, batch] on the PE (identity-matmul transpose) at a
cost of ~3H/128 extra PE ops per step. tanh(v) = 2*sigmoid(2v)-1 keeps
the ACT engine on a single function table. The masked time-sum
accumulates on-chip in fp32; the host just transposes/concats the eight
per-core [3584, 16] outputs.
"""

import sys

if "/opt/trn_rl_repo" not in sys.path:
    sys.path.insert(0, "/opt/trn_rl_repo")

import numpy as np

NCORES = 8
B = 128
BC = 16                                   # batch rows per core
HS = (512, 1024, 2048)
INS = (512, 512, 1024)
KIN = tuple(i // 128 for i in INS)        # input-dim 128-chunks: 4, 4, 8
KH = tuple(h // 128 for h in HS)          # hidden-dim 128-chunks: 4, 8, 16
RES_COLS = 3584                           # resident Whh2^T columns
STR_CH = 256                              # streamed-column chunk size


def _split_multiwaits(nc):
    """walrus in this container rejects >1 sync-wait per instruction; hoist
    extras into standalone nop-waits on the same engine (per-engine program
    order is preserved, so this is semantically identical)."""
    import concourse.mybir as mybir

    for f in nc.m.functions:
        for bb in f.blocks:
            old = list(bb.instructions)
            if not any(
                ins.sync_info is not None and len(ins.sync_info.on_wait) > 1
                for ins in old
            ):
                continue
            new = []
            for ins in old:
                si = ins.sync_info
                if si is not None and len(si.on_wait) > 1:
                    waits = list(si.on_wait)
                    for j, w in enumerate(waits[:-1]):
                        new.append(
                            mybir.InstNoOp(
                                name=f"{ins.name}-ws{j}",
                                engine=ins.engine,
                                sync_info=mybir.SyncInfo(on_wait=[w], on_update=[]),
                            )
                        )
                    ins.sync_info = mybir.SyncInfo(
                        on_wait=[waits[-1]], on_update=list(si.on_update)
                    )
                new.append(ins)
            bb.instructions = new


def _build(T, waitfix=True):
    import concourse.bass as bass
    import concourse.mybir as mybir
    import concourse.tile as tile
    from concourse.masks import make_identity

    assert T % 8 == 0
    f32 = mybir.dt.float32
    bf16 = mybir.dt.bfloat16
    Sig = mybir.ActivationFunctionType.Sigmoid
    ADD = mybir.AluOpType.add
    MUL = mybir.AluOpType.mult
    NT = T * BC

    nc = bass.Bass(num_devices=NCORES)

    xT_d = nc.dram_tensor("xT", [KIN[0], 128, NT], bf16, kind="ExternalInput")
    mrep_d = nc.dram_tensor("mrep", [T, 128, BC], f32, kind="ExternalInput")
    wih_d, whh_d, bzr_d, bn2_d = [], [], [], []
    for l in range(3):
        wih_d.append(nc.dram_tensor(f"wih{l}", [KIN[l], 128, 3 * HS[l]], bf16,
                                    kind="ExternalInput"))
        whh_d.append(nc.dram_tensor(f"whh{l}", [KH[l], 128, 3 * HS[l]], bf16,
                                    kind="ExternalInput"))
        bzr_d.append(nc.dram_tensor(f"bzr{l}", [2 * HS[l] // 128, 128], f32,
                                    kind="ExternalInput"))
        bn2_d.append(nc.dram_tensor(f"bn2{l}", [HS[l] // 128, 128], f32,
                                    kind="ExternalInput"))
    out_d = nc.dram_tensor("out", [3584, BC], f32, kind="ExternalOutput")
    gi_d = [nc.dram_tensor(f"gi{l}_sc", [T, BC, 3 * HS[l]], bf16) for l in range(3)]

    with tile.TileContext(nc) as tc:
        with (
            tc.tile_pool(name="wp", bufs=1) as wp,
            tc.tile_pool(name="pb", space="PSUM", bufs=2) as pb,
            tc.tile_pool(name="pz", space="PSUM", bufs=2) as pz,
            tc.tile_pool(name="pnp", space="PSUM", bufs=2) as pnp,
            tc.tile_pool(name="pg", space="PSUM", bufs=2) as pg,
        ):
            ident = wp.tile([BC, BC], bf16, name="ident")
            make_identity(nc, ident[:])
            bzr_s, bn2_s, accs = [], [], []
            for l in range(3):
                t_ = wp.tile([128, 2 * HS[l] // 128], f32, name=f"bzr_s{l}")
                nc.sync.dma_start(out=t_[:], in_=bzr_d[l][:].rearrange("m p -> p m"))
                bzr_s.append(t_)
                t2 = wp.tile([128, HS[l] // 128], f32, name=f"bn2_s{l}")
                nc.sync.dma_start(out=t2[:], in_=bn2_d[l][:].rearrange("m p -> p m"))
                bn2_s.append(t2)
                a_ = wp.tile([128, KH[l], BC], f32, name=f"acc{l}")
                nc.vector.memset(a_[:], 0.0)
                accs.append(a_)

            def load_w(pool, dram, kc, cols, name, col0=0):
                t_ = pool.tile([128, kc, cols], bf16, name=name)
                nc.sync.dma_start(
                    out=t_[:],
                    in_=dram[:, :, col0 : col0 + cols].rearrange("k p m -> p k m"),
                )
                return t_

            def bulk_gi(l, lhs_sb, wih_s, pool):
                # gi[(t,b), gate] blocks -> DRAM [T, BC, 3H] bf16.
                # stationary: input-series chunk [128, 128 (t,b)-cols];
                # moving: Wih^T columns.
                H3 = 3 * HS[l]
                for tb in range(NT // 128):
                    t0 = tb * 128 // BC
                    for c0 in range(0, H3, 512):
                        ps = pb.tile([128, 512], f32, tag="pblk")
                        for k in range(KIN[l]):
                            nc.tensor.matmul(
                                ps[:],
                                lhs_sb[:, k, tb * 128 : (tb + 1) * 128],
                                wih_s[:, k, c0 : c0 + 512],
                                start=(k == 0),
                                stop=(k == KIN[l] - 1),
                            )
                        stg = pool.tile([128, 512], bf16, tag="stg", bufs=3)
                        nc.vector.tensor_copy(stg[:], ps[:])
                        nc.sync.dma_start(
                            out=gi_d[l][t0 : t0 + 128 // BC, :, c0 : c0 + 512]
                            .rearrange("t b n -> (t b) n"),
                            in_=stg[:],
                        )

            def recurrence(l, whh_s, h_ser, res_cols, str_dram, pool):
                kh = KH[l]
                nzr = 2 * HS[l] // 128
                nn_ = HS[l] // 128
                h3c = 3 * HS[l] // 128
                acc = accs[l]
                h_f = None
                h_bf = None
                wstr = {}

                def w_ap(t, m, k):
                    col = m * 128
                    if col < res_cols:
                        return whh_s[:, k, col : col + 128]
                    j = (col - res_cols) // STR_CH
                    if (t, j) not in wstr:
                        st = pool.tile([128, kh, STR_CH], bf16, tag=f"wstr{l}", bufs=3)
                        nc.sync.dma_start(
                            out=st[:],
                            in_=str_dram[
                                :, :, res_cols + j * STR_CH : res_cols + (j + 1) * STR_CH
                            ].rearrange("k p m -> p k m"),
                        )
                        wstr[(t, j)] = st
                    rem = (col - res_cols) % STR_CH
                    return wstr[(t, j)][:, k, rem : rem + 128]

                for t in range(T):
                    mk = pool.tile([128, BC], f32, tag=f"mk{l}", bufs=3)
                    nc.sync.dma_start(out=mk[:], in_=mrep_d[t])
                    # gi slice -> PE transpose into [gate, b] bf16 psum
                    gis = pool.tile([BC, 3 * HS[l]], bf16, tag=f"gis{l}", bufs=2)
                    nc.sync.dma_start(out=gis[:], in_=gi_d[l][t])
                    gps = pg.tile([128, h3c, BC], bf16, tag="pgi")
                    for m in range(h3c):
                        nc.tensor.matmul(
                            gps[:, m, :],
                            gis[:, m * 128 : (m + 1) * 128],
                            ident[:],
                            is_transpose=True,
                        )
                    # only one PSUM operand allowed per DVE inst; stage in SBUF
                    gsb = pool.tile([128, h3c, BC], bf16, tag=f"gsb{l}", bufs=2)
                    nc.vector.tensor_copy(gsb[:], gps[:])
                    # ---- z, r ----
                    pre = pool.tile([128, nzr, BC], f32, tag=f"pre{l}", bufs=2)
                    if t > 0:
                        ps = pz.tile([128, nzr, BC], f32, tag="pzr")
                        for m in range(nzr):
                            for k in range(kh):
                                nc.tensor.matmul(
                                    ps[:, m, :], w_ap(t, m, k), h_bf[:, k, :],
                                    start=(k == 0), stop=(k == kh - 1),
                                )
                        for m in range(nzr):
                            nc.vector.scalar_tensor_tensor(
                                pre[:, m, :], ps[:, m, :],
                                bzr_s[l][:, m : m + 1], gsb[:, m, :], ADD, ADD,
                            )
                    else:
                        for m in range(nzr):
                            nc.vector.tensor_scalar_add(
                                pre[:, m, :], gsb[:, m, :], bzr_s[l][:, m : m + 1]
                            )
                    zr = pool.tile([128, nzr, BC], f32, tag=f"zr{l}", bufs=2)
                    nc.scalar.activation(zr[:], pre[:], Sig)
                    # ---- n ----
                    pre_n = pool.tile([128, nn_, BC], f32, tag=f"pren{l}", bufs=2)
                    if t > 0:
                        rh = pool.tile([128, kh, BC], bf16, tag=f"rh{l}", bufs=2)
                        nc.vector.tensor_mul(rh[:], zr[:, nn_ : 2 * nn_, :], h_f[:])
                        ps2 = pnp.tile([128, nn_, BC], f32, tag="pn")
                        for m in range(nn_):
                            for k in range(kh):
                                nc.tensor.matmul(
                                    ps2[:, m, :], w_ap(t, nzr + m, k), rh[:, k, :],
                                    start=(k == 0), stop=(k == kh - 1),
                                )
                        nc.vector.tensor_add(
                            pre_n[:], ps2[:], gsb[:, nzr : nzr + nn_, :]
                        )
                    else:
                        nc.vector.tensor_copy(pre_n[:], gsb[:, nzr : nzr + nn_, :])
                    s_t = pool.tile([128, nn_, BC], f32, tag=f"st{l}", bufs=2)
                    for m in range(nn_):
                        nc.scalar.activation(
                            s_t[:, m, :], pre_n[:, m, :], Sig,
                            bias=bn2_s[l][:, m : m + 1], scale=2.0,
                        )
                    n_t = pool.tile([128, nn_, BC], f32, tag=f"nt{l}", bufs=2)
                    nc.vector.tensor_scalar(n_t[:], s_t[:], 2.0, -1.0, MUL, ADD)
                    # ---- h update ----
                    d = pool.tile([128, nn_, BC], f32, tag=f"d{l}", bufs=2)
                    if t > 0:
                        nc.vector.tensor_sub(d[:], h_f[:], n_t[:])
                    else:
                        nc.vector.tensor_scalar(d[:], n_t[:], -1.0, None, MUL)
                    h_new = pool.tile([128, nn_, BC], f32, tag=f"hf{l}", bufs=2)
                    nc.vector.tensor_mul(h_new[:], zr[:, 0:nn_, :], d[:])
                    nc.vector.tensor_add(h_new[:], h_new[:], n_t[:])
                    h_f = h_new
                    if h_ser is not None:
                        nc.vector.tensor_copy(h_ser[:, :, t * BC : (t + 1) * BC], h_f[:])
                        h_bf = h_ser[:, :, t * BC : (t + 1) * BC]
                    else:
                        hb = pool.tile([128, kh, BC], bf16, tag=f"hb{l}", bufs=2)
                        nc.vector.tensor_copy(hb[:], h_f[:])
                        h_bf = hb
                    # ---- masked accumulate ----
                    am = pool.tile([128, nn_, BC], f32, tag=f"am{l}", bufs=2)
                    for k in range(nn_):
                        nc.vector.tensor_mul(am[:, k, :], h_f[:, k, :], mk[:])
                    nc.vector.tensor_add(acc[:], acc[:], am[:])

            # ---------------- phases ----------------
            with tc.tile_pool(name="p_b0", bufs=1) as p_b0:
                xT = p_b0.tile([128, KIN[0], NT], bf16, name="xT_s")
                nc.sync.dma_start(out=xT[:], in_=xT_d[:].rearrange("k p m -> p k m"))
                wih0 = load_w(p_b0, wih_d[0], KIN[0], 3 * HS[0], "wih0_s")
                bulk_gi(0, xT, wih0, p_b0)

            with tc.tile_pool(name="p_s0", bufs=1) as p_s0:
                h0_ser = p_s0.tile([128, KH[0], NT], bf16, name="h0_ser")
                with tc.tile_pool(name="p_r0", bufs=1) as p_r0:
                    whh0 = load_w(p_r0, whh_d[0], KH[0], 3 * HS[0], "whh0_s")
                    recurrence(0, whh0, h0_ser, 3 * HS[0], None, p_r0)
                with tc.tile_pool(name="p_b1", bufs=1) as p_b1:
                    wih1 = load_w(p_b1, wih_d[1], KIN[1], 3 * HS[1], "wih1_s")
                    bulk_gi(1, h0_ser, wih1, p_b1)

            with tc.tile_pool(name="p_s1", bufs=1) as p_s1:
                h1_ser = p_s1.tile([128, KH[1], NT], bf16, name="h1_ser")
                with tc.tile_pool(name="p_r1", bufs=1) as p_r1:
                    whh1 = load_w(p_r1, whh_d[1], KH[1], 3 * HS[1], "whh1_s")
                    recurrence(1, whh1, h1_ser, 3 * HS[1], None, p_r1)
                with tc.tile_pool(name="p_b2", bufs=1) as p_b2:
                    wih2 = load_w(p_b2, wih_d[2], KIN[2], 3 * HS[2], "wih2_s")
                    bulk_gi(2, h1_ser, wih2, p_b2)

            with tc.tile_pool(name="p_r2", bufs=1) as p_r2:
                whh2r = load_w(p_r2, whh_d[2], KH[2], RES_COLS, "whh2r_s")
                recurrence(2, whh2r, None, RES_COLS, whh_d[2], p_r2)

            nc.sync.dma_start(
                out=out_d[0:512, :].rearrange("(k p) b -> p k b", p=128),
                in_=accs[0][:],
            )
            nc.sync.dma_start(
                out=out_d[512:1536, :].rearrange("(k p) b -> p k b", p=128),
                in_=accs[1][:],
            )
            nc.sync.dma_start(
                out=out_d[1536:3584, :].rearrange("(k p) b -> p k b", p=128),
                in_=accs[2][:],
            )

    if waitfix:
        _split_multiwaits(nc)
    return nc


# ---------------- host side ----------------

def _prep_core_inputs(c, x, mask, params, T):
    import ml_dtypes

    bf = ml_dtypes.bfloat16
    b0 = c * BC
    m = {}
    xt = x[b0 : b0 + BC, :T, :].transpose(2, 1, 0).reshape(INS[0], T * BC)
    m["xT"] = np.ascontiguousarray(xt.reshape(KIN[0], 128, T * BC)).astype(bf)
    m["mrep"] = np.ascontiguousarray(
        np.broadcast_to(mask[b0 : b0 + BC, :T].T[:, None, :], (T, 128, BC))
    ).astype(np.float32)
    for l, (Wih, Whh, bias) in enumerate(params):
        H = HS[l]
        m[f"wih{l}"] = np.ascontiguousarray(
            Wih.T.reshape(KIN[l], 128, 3 * H)
        ).astype(bf)
        m[f"whh{l}"] = np.ascontiguousarray(
            Whh.T.reshape(KH[l], 128, 3 * H)
        ).astype(bf)
        m[f"bzr{l}"] = np.ascontiguousarray(
            bias[: 2 * H].reshape(2 * H // 128, 128)
        ).astype(np.float32)
        m[f"bn2{l}"] = np.ascontiguousarray(
            (2.0 * bias[2 * H :]).reshape(H // 128, 128)
        ).astype(np.float32)
    return m


_cache = {}


def kernel(x, mask, Wih0, Whh0, b0, Wih1, Whh1, b1, Wih2, Whh2, b2):
    from concourse.bass_utils import run_bass_kernel_spmd

    x = np.asarray(x, np.float32)
    mask = np.asarray(mask, np.float32)
    T = x.shape[1]
    if T not in _cache:
        _cache[T] = _build(T)
    nc = _cache[T]
    params = [
        (np.asarray(Wih0, np.float32), np.asarray(Whh0, np.float32),
         np.asarray(b0, np.float32)),
        (np.asarray(Wih1, np.float32), np.asarray(Whh1, np.float32),
         np.asarray(b1, np.float32)),
        (np.asarray(Wih2, np.float32), np.asarray(Whh2, np.float32),
         np.asarray(b2, np.float32)),
    ]
    in_maps = [_prep_core_inputs(c, x, mask, params, T) for c in range(NCORES)]
    res = run_bass_kernel_spmd(nc, in_maps, core_ids=list(range(NCORES)))
    out = np.zeros((B, 3584), np.float32)
    for c in range(NCORES):
        out[c * BC : (c + 1) * BC, :] = res.results[c]["out"].T
    return out

